# revision 30
# baseline (speedup 1.0000x reference)
"""MoE layer (B=4,S=2048,H=1024,I=4096,E=8,K=2) as a Bass/Tile kernel on 8
Trainium2 NeuronCores.

Strategy (expert parallelism, per the sharding hint):
  - Host computes the routing decision (softmax + top-2 over E=8) in fp32 as
    the *sharding* step, and dispatches each expert's tokens to the core that
    owns that expert (all-to-all realized at input-sharding time).
  - Core e holds expert e's FFN weights (bf16) and runs the expert FFN
    (x @ up_w -> +up_b -> gelu -> @ down_w -> +down_b -> * combine_weight)
    over its dispatched tokens (fixed capacity C, zero-padded).
  - Each core also runs the (replicated-weight) router on its 1/8 slice of
    tokens on the PE (bf16 matmul, fp32 softmax) to produce the expert-usage
    partial sums for the aux outputs.
  - Host combine: scatter-add per-core compact outputs back to [B,S,H],
    usage = mean of device partials, loss = E * sum(usage^2),
    top_expert = routing decision.

Self-contained: hardcodes shapes from the problem spec.
"""
import os
import numpy as np
import ml_dtypes
from contextlib import ExitStack

from concourse import bass, bacc, tile, mybir
from concourse.bass_utils import run_bass_kernel_spmd

BF16 = ml_dtypes.bfloat16


def _ensure_axon_hooks_shim():
    """bass_utils' axon trace path imports antenv.axon_hooks, which is absent
    in this image; provide a no-op shim so a stray BASS_TRACE=1 in the
    environment degrades to 'no trace' instead of crashing."""
    try:
        import antenv.axon_hooks  # noqa: F401
    except Exception:
        import sys
        import types
        try:
            import antenv
        except Exception:
            return
        mod = types.ModuleType("antenv.axon_hooks")
        _h = [None]
        mod.set_axon_ntff_profile_hook = lambda h: _h.__setitem__(0, h)
        mod.get_axon_ntff_profile_hook = lambda: _h[0]
        sys.modules["antenv.axon_hooks"] = mod
        antenv.axon_hooks = mod


_ensure_axon_hooks_shim()

# Problem shapes
B, S, H, I, E, TOPK = 4, 2048, 1024, 4096, 8, 2
TOK = B * S           # 8192 tokens
TPC = TOK // E        # 1024 tokens per core for the router slice
CAP = 2184            # per-expert token capacity (max observed count is 2182)
CHUNK = 512           # moving-operand/PSUM-bank tile over tokens

HT = H // 128         # 8  H tiles
IT = I // 128         # 32 I tiles

_f32 = mybir.dt.float32
_bf16 = mybir.dt.bfloat16

_COMPILED = None

HALF = CAP // 2       # tokens per half-pass


def _chunks(total):
    out = []
    off = 0
    while off < total:
        n = min(CHUNK, total - off)
        out.append((off, n))
        off += n
    return out


def _build():
    """Build + compile the single-core SPMD program (same program on all 8
    cores; per-core data differs via in_maps)."""
    nc = bacc.Bacc("TRN2", target_bir_lowering=False, debug=False)

    # ---- DRAM I/O ----
    xt_ffn = nc.dram_tensor("xt_ffn", [H, CAP], _bf16, kind="ExternalInput")
    up_w = nc.dram_tensor("up_w", [H, I], _bf16, kind="ExternalInput")
    down_w = nc.dram_tensor("down_w", [I, H], _bf16, kind="ExternalInput")
    up_b = nc.dram_tensor("up_b", [128, IT], _f32, kind="ExternalInput")
    w_bc = nc.dram_tensor("w_bc", [128, CAP + HT], _f32, kind="ExternalInput")
    xt_rtr = nc.dram_tensor("xt_rtr", [H, TPC], _bf16, kind="ExternalInput")
    gate_w = nc.dram_tensor("gate_w", [H, E], _bf16, kind="ExternalInput")

    y_out = nc.dram_tensor("y_out", [H, CAP], _f32, kind="ExternalOutput")
    usage_p = nc.dram_tensor("usage_p", [E, 1], _f32, kind="ExternalOutput")

    with tile.TileContext(nc) as tc:
        with ExitStack() as ctx:
            const_pool = ctx.enter_context(tc.tile_pool(name="const", bufs=1))
            up_pool = ctx.enter_context(tc.tile_pool(name="up", bufs=8))
            dwn_pool = ctx.enter_context(tc.tile_pool(name="dwn", bufs=2))
            xc_pool = ctx.enter_context(tc.tile_pool(name="xc", bufs=2))
            g_pool = ctx.enter_context(tc.tile_pool(name="g", bufs=1))
            y_pool = ctx.enter_context(tc.tile_pool(name="y", bufs=2))
            rtr_pool = ctx.enter_context(tc.tile_pool(name="rtr", bufs=1))
            sm_pool = ctx.enter_context(tc.tile_pool(name="sm", bufs=4))
            ps_pool = ctx.enter_context(
                tc.tile_pool(name="ps", bufs=2, space="PSUM"))
            psl_pool = ctx.enter_context(
                tc.tile_pool(name="psl", bufs=1, space="PSUM"))
            psu_pool = ctx.enter_context(
                tc.tile_pool(name="psu", bufs=1, space="PSUM"))

            # ---- constants ----
            ones = const_pool.tile([128, 1], _f32)
            nc.vector.memset(ones[:, :], 1.0)
            gw_sb = const_pool.tile([128, HT, E], _bf16)
            nc.scalar.dma_start(
                out=gw_sb[:, :, :],
                in_=gate_w[:, :].rearrange("(a p) e -> p a e", p=128))
            upb_sb = const_pool.tile([128, IT], _f32)
            nc.scalar.dma_start(out=upb_sb[:, :], in_=up_b[:, :])
            # ---- router over this core's 1/8 token slice (bf16 matmul,
            # fp32 softmax; only feeds expert_usage). Its input streams in on
            # the scalar queue while the FFN runs; its matmuls are emitted in
            # the natural mm1->mm2 bubble of the first FFN half-pass so they
            # never delay the FFN stream. ----
            ps_u = psu_pool.tile([E, 1], _f32)
            nt = TPC // 128
            probs_all = const_pool.tile([128, nt, E], _f32)
            xr = rtr_pool.tile([128, HT, TPC], _bf16)

            def emit_xr_load():
                nc.scalar.dma_start(
                    out=xr[:, :, :],
                    in_=xt_rtr[:, :].rearrange("(a p) n -> p a n", p=128))

            def emit_router_logits():
                for t in range(nt):
                    ps_l = psl_pool.tile([128, E], _f32, tag="psl")
                    for k in range(HT):
                        nc.tensor.matmul(
                            ps_l[:, :], xr[:, k, t * 128:(t + 1) * 128],
                            gw_sb[:, k, :],
                            start=(k == 0), stop=(k == HT - 1))
                    mneg = sm_pool.tile([128, 1], _f32)
                    nc.vector.tensor_reduce(
                        mneg[:, :], ps_l[:, :], axis=mybir.AxisListType.X,
                        op=mybir.AluOpType.max, negate=True)
                    expv = sm_pool.tile([128, E], _f32)
                    ssum = sm_pool.tile([128, 1], _f32)
                    nc.scalar.activation(
                        expv[:, :], ps_l[:, :],
                        mybir.ActivationFunctionType.Exp,
                        bias=mneg[:, 0:1], scale=1.0, accum_out=ssum[:, 0:1])
                    rs = sm_pool.tile([128, 1], _f32)
                    nc.vector.reciprocal(rs[:, :], ssum[:, :])
                    nc.vector.tensor_scalar_mul(
                        probs_all[:, t, :], expv[:, :], rs[:, 0:1])

            def emit_router_usage():
                # separate pass: by now the softmax chain has long finished,
                # so the PE never waits on it
                for t in range(nt):
                    nc.tensor.matmul(
                        ps_u[:, :], probs_all[:, t, :], ones[:, :],
                        start=(t == 0), stop=(t == nt - 1),
                        skip_group_check=True)
                u_sb = sm_pool.tile([E, 1], _f32)
                nc.vector.tensor_copy(u_sb[:, :], ps_u[:, :])
                nc.sync.dma_start(out=usage_p[:, :], in_=u_sb[:, :])

            w_sb = const_pool.tile([128, CAP + HT], _f32)
            nc.scalar.dma_start(out=w_sb[:, :], in_=w_bc[:, :])

            # ---- expert FFN over two half-passes of HALF tokens ----
            # Loop order maximizes stationary-weight reuse (one LDWEIGHTS per
            # 3 matmuls) while keeping g for one half resident in SBUF.
            def load_xc(hoff):
                # split the half's activation load into 4 pieces across two
                # DMA queues so the first matmuls can start early
                xc = xc_pool.tile([128, HT, HALF], _bf16, tag="xc")
                for piece in range(4):
                    eng = nc.sync if piece == 0 else nc.gpsimd
                    eng.dma_start(
                        out=xc[:, piece * 2:(piece + 1) * 2, :],
                        in_=xt_ffn[piece * 256:(piece + 1) * 256,
                                   hoff:hoff + HALF].rearrange(
                            "(a p) n -> p a n", p=128))
                return xc

            xc_next = load_xc(0)
            for hoff in (0, HALF):
                cs = _chunks(HALF)
                xc = xc_next
                g = g_pool.tile([128, IT, HALF], _bf16)
                # mm1: g[i] = gelu(up_w[:, i].T @ xc + up_b[i]); weight tiles
                # streamed in pairs of I-tiles for 512B DMA segments
                for ip in range(IT // 2):
                    if hoff == 0 and ip == 4:
                        emit_xr_load()
                    upt = up_pool.tile([128, HT, 256], _bf16)
                    eng = nc.sync if ip % 2 == 0 else nc.gpsimd
                    eng.dma_start(
                        out=upt[:, :, :],
                        in_=up_w[:, ip * 256:(ip + 1) * 256].rearrange(
                            "(a p) n -> p a n", p=128))
                    for sub in range(2):
                        i = ip * 2 + sub
                        ps1 = ps_pool.tile([128, HALF], _f32, tag="mmps")
                        for k in range(HT):
                            for (off, ncols) in cs:
                                nc.tensor.matmul(
                                    ps1[:, off:off + ncols],
                                    upt[:, k, sub * 128:(sub + 1) * 128],
                                    xc[:, k, off:off + ncols],
                                    start=(k == 0), stop=(k == HT - 1))
                        nc.scalar.activation(
                            g[:, i, :], ps1[:, :],
                            mybir.ActivationFunctionType.Gelu,
                            bias=upb_sb[:, i:i + 1], scale=1.0)
                if hoff == 0:
                    # fills the PE bubble while mm2 waits on the last gelu
                    emit_router_logits()
                # mm2: yT[h] = (down_w[:, h].T @ g + down_b[h]) * w
                for hp in range(HT // 2):
                    if hp == 1 and hoff == 0:
                        xc_next = load_xc(HALF)
                        emit_router_usage()
                    dwn = dwn_pool.tile([128, IT, 256], _bf16)
                    nc.scalar.dma_start(
                        out=dwn[:, :, :],
                        in_=down_w[:, hp * 256:(hp + 1) * 256].rearrange(
                            "(a p) n -> p a n", p=128))
                    for sub in range(2):
                        h = hp * 2 + sub
                        ps2 = ps_pool.tile([128, HALF], _f32, tag="mmps")
                        for i in range(IT):
                            for (off, ncols) in cs:
                                nc.tensor.matmul(
                                    ps2[:, off:off + ncols],
                                    dwn[:, i, sub * 128:(sub + 1) * 128],
                                    g[:, i, off:off + ncols],
                                    start=(i == 0), stop=(i == IT - 1))
                        yt = y_pool.tile([128, HALF], _f32)
                        nc.vector.scalar_tensor_tensor(
                            yt[:, :], ps2[:, :],
                            w_sb[:, CAP + h:CAP + h + 1],
                            w_sb[:, hoff:hoff + HALF],
                            op0=mybir.AluOpType.add,
                            op1=mybir.AluOpType.mult)
                        nc.gpsimd.dma_start(
                            out=y_out[h * 128:(h + 1) * 128,
                                      hoff:hoff + HALF],
                            in_=yt[:, :])


    nc.compile()
    return nc


def _get_compiled():
    global _COMPILED
    if _COMPILED is None:
        _COMPILED = _build()
    return _COMPILED


def _route(x2d, gate_w):
    """fp32 routing identical in structure to the jax reference."""
    logits = x2d @ gate_w                       # [TOK, E] fp32
    m = logits.max(-1, keepdims=True)
    p = np.exp(logits - m, dtype=np.float32)
    p = p / p.sum(-1, keepdims=True, dtype=np.float32)
    i1 = np.argmax(p, axis=-1)
    rows = np.arange(p.shape[0])
    p1 = p[rows, i1]
    pm = p.copy()
    pm[rows, i1] = -np.inf
    i2 = np.argmax(pm, axis=-1)
    p2 = p[rows, i2]
    den = p1 + p2
    return i1.astype(np.int32), i2.astype(np.int32), (p1 / den).astype(
        np.float32), (p2 / den).astype(np.float32)


def kernel(x, gate_w, up_w, up_b, down_w, down_b):
    x = np.asarray(x, dtype=np.float32)
    gate_w = np.asarray(gate_w, dtype=np.float32)
    up_w = np.asarray(up_w, dtype=np.float32)
    up_b = np.asarray(up_b, dtype=np.float32)
    down_w = np.asarray(down_w, dtype=np.float32)
    down_b = np.asarray(down_b, dtype=np.float32)

    x2d = x.reshape(TOK, H)
    i1, i2, w1, w2 = _route(x2d, gate_w)

    # dispatch lists per expert
    sel_idx, sel_w, overflow = [], [], []
    for e in range(E):
        sel = np.where((i1 == e) | (i2 == e))[0]
        w = np.where(i1[sel] == e, w1[sel], w2[sel]).astype(np.float32)
        if len(sel) > CAP:  # safety net; never hit for the spec'd inputs
            overflow.append((e, sel[CAP:], w[CAP:]))
            sel, w = sel[:CAP], w[:CAP]
        sel_idx.append(sel)
        sel_w.append(w)

    nc = _get_compiled()

    in_maps = []
    for e in range(E):
        sel, w = sel_idx[e], sel_w[e]
        n = len(sel)
        xt = np.zeros((H, CAP), dtype=BF16)
        xt[:, :n] = x2d[sel].T.astype(BF16)
        wb = np.zeros((128, CAP + HT), dtype=np.float32)
        wb[:, :n] = w[None, :]
        wb[:, CAP:] = down_b[e].reshape(HT, 128).T
        in_maps.append({
            "xt_ffn": xt,
            "up_w": up_w[e].astype(BF16),
            "down_w": down_w[e].astype(BF16),
            "up_b": np.ascontiguousarray(up_b[e].reshape(IT, 128).T),
            "w_bc": wb,
            "xt_rtr": np.ascontiguousarray(x2d[e * TPC:(e + 1) * TPC].T).astype(BF16),
            "gate_w": gate_w.astype(BF16),
        })

    res = run_bass_kernel_spmd(nc, in_maps, core_ids=list(range(E)))

    out = np.zeros((TOK, H), dtype=np.float32)
    usage = np.zeros(E, dtype=np.float32)
    for e in range(E):
        y = res.results[e]["y_out"]            # [H, CAP] f32, already *w
        n = len(sel_idx[e])
        out[sel_idx[e]] += y[:, :n].T
        usage += res.results[e]["usage_p"][:, 0]
    usage /= np.float32(TOK)

    # overflow tokens (only if capacity were ever exceeded): host fp32 FFN
    for (e, sel, w) in overflow:
        h1 = x2d[sel] @ up_w[e] + up_b[e]
        from scipy.special import erf
        g = 0.5 * h1 * (1.0 + erf(h1 / np.sqrt(2.0)))
        out[sel] += w[:, None] * (g.astype(np.float32) @ down_w[e] + down_b[e])

    loss = np.float32(E) * np.sum(usage.astype(np.float32) ** 2)
    top_expert = i1.reshape(B, S)
    return (out.reshape(B, S, H), np.float32(loss), usage,
            top_expert.astype(np.int32))


# revision 31
# speedup vs baseline: 1.0073x; 1.0073x over previous
"""MoE layer (B=4,S=2048,H=1024,I=4096,E=8,K=2) as a Bass/Tile kernel on 8
Trainium2 NeuronCores.

Strategy (expert parallelism, per the sharding hint):
  - Host computes the routing decision (softmax + top-2 over E=8) in fp32 as
    the *sharding* step, and dispatches each expert's tokens to the core that
    owns that expert (all-to-all realized at input-sharding time).
  - Core e holds expert e's FFN weights (bf16) and runs the expert FFN
    (x @ up_w -> +up_b -> gelu -> @ down_w -> +down_b -> * combine_weight)
    over its dispatched tokens (fixed capacity C, zero-padded).
  - Each core also runs the (replicated-weight) router on its 1/8 slice of
    tokens on the PE (bf16 matmul, fp32 softmax) to produce the expert-usage
    partial sums for the aux outputs.
  - Host combine: scatter-add per-core compact outputs back to [B,S,H],
    usage = mean of device partials, loss = E * sum(usage^2),
    top_expert = routing decision.

Self-contained: hardcodes shapes from the problem spec.
"""
import os
import numpy as np
import ml_dtypes
from contextlib import ExitStack

from concourse import bass, bacc, tile, mybir
from concourse.bass_utils import run_bass_kernel_spmd

BF16 = ml_dtypes.bfloat16


def _ensure_axon_hooks_shim():
    """bass_utils' axon trace path imports antenv.axon_hooks, which is absent
    in this image; provide a no-op shim so a stray BASS_TRACE=1 in the
    environment degrades to 'no trace' instead of crashing."""
    try:
        import antenv.axon_hooks  # noqa: F401
    except Exception:
        import sys
        import types
        try:
            import antenv
        except Exception:
            return
        mod = types.ModuleType("antenv.axon_hooks")
        _h = [None]
        mod.set_axon_ntff_profile_hook = lambda h: _h.__setitem__(0, h)
        mod.get_axon_ntff_profile_hook = lambda: _h[0]
        sys.modules["antenv.axon_hooks"] = mod
        antenv.axon_hooks = mod


_ensure_axon_hooks_shim()

# Problem shapes
B, S, H, I, E, TOPK = 4, 2048, 1024, 4096, 8, 2
TOK = B * S           # 8192 tokens
TPC = TOK // E        # 1024 tokens per core for the router slice
CAP = 2184            # per-expert token capacity (max observed count is 2182)
CHUNK = 512           # moving-operand/PSUM-bank tile over tokens

HT = H // 128         # 8  H tiles
IT = I // 128         # 32 I tiles

_f32 = mybir.dt.float32
_bf16 = mybir.dt.bfloat16

_COMPILED = None

HALF = CAP // 2       # tokens per half-pass


def _chunks(total):
    out = []
    off = 0
    while off < total:
        n = min(CHUNK, total - off)
        out.append((off, n))
        off += n
    return out


def _build():
    """Build + compile the single-core SPMD program (same program on all 8
    cores; per-core data differs via in_maps)."""
    nc = bacc.Bacc("TRN2", target_bir_lowering=False, debug=False)

    # ---- DRAM I/O ----
    xt_ffn = nc.dram_tensor("xt_ffn", [H, CAP], _bf16, kind="ExternalInput")
    up_w = nc.dram_tensor("up_w", [H, I], _bf16, kind="ExternalInput")
    down_w = nc.dram_tensor("down_w", [I, H], _bf16, kind="ExternalInput")
    up_b = nc.dram_tensor("up_b", [128, IT], _f32, kind="ExternalInput")
    w_bc = nc.dram_tensor("w_bc", [128, CAP + HT], _f32, kind="ExternalInput")
    xt_rtr = nc.dram_tensor("xt_rtr", [H, TPC], _bf16, kind="ExternalInput")
    gate_w = nc.dram_tensor("gate_w", [H, E], _bf16, kind="ExternalInput")

    y_out = nc.dram_tensor("y_out", [H, CAP], _f32, kind="ExternalOutput")
    usage_p = nc.dram_tensor("usage_p", [E, 1], _f32, kind="ExternalOutput")

    with tile.TileContext(nc) as tc:
        with ExitStack() as ctx:
            const_pool = ctx.enter_context(tc.tile_pool(name="const", bufs=1))
            up_pool = ctx.enter_context(tc.tile_pool(name="up", bufs=8))
            dwn_pool = ctx.enter_context(tc.tile_pool(name="dwn", bufs=2))
            # bufs=1: the slot dependency keeps half-1's activation load out of
            # the startup window (Tile would otherwise hoist it ahead of the
            # first weight pairs); it still prefetches ~200us before use
            xc_pool = ctx.enter_context(tc.tile_pool(name="xc", bufs=1))
            g_pool = ctx.enter_context(tc.tile_pool(name="g", bufs=1))
            y_pool = ctx.enter_context(tc.tile_pool(name="y", bufs=2))
            rtr_pool = ctx.enter_context(tc.tile_pool(name="rtr", bufs=1))
            sm_pool = ctx.enter_context(tc.tile_pool(name="sm", bufs=4))
            ps_pool = ctx.enter_context(
                tc.tile_pool(name="ps", bufs=2, space="PSUM"))
            psl_pool = ctx.enter_context(
                tc.tile_pool(name="psl", bufs=1, space="PSUM"))
            psu_pool = ctx.enter_context(
                tc.tile_pool(name="psu", bufs=1, space="PSUM"))

            # ---- constants ----
            ones = const_pool.tile([128, 1], _f32)
            nc.vector.memset(ones[:, :], 1.0)
            gw_sb = const_pool.tile([128, HT, E], _bf16)
            nc.scalar.dma_start(
                out=gw_sb[:, :, :],
                in_=gate_w[:, :].rearrange("(a p) e -> p a e", p=128))
            upb_sb = const_pool.tile([128, IT], _f32)
            nc.scalar.dma_start(out=upb_sb[:, :], in_=up_b[:, :])
            # ---- router over this core's 1/8 token slice (bf16 matmul,
            # fp32 softmax; only feeds expert_usage). Its input streams in on
            # the scalar queue while the FFN runs; its matmuls are emitted in
            # the natural mm1->mm2 bubble of the first FFN half-pass so they
            # never delay the FFN stream. ----
            ps_u = psu_pool.tile([E, 1], _f32)
            nt = TPC // 128
            probs_all = const_pool.tile([128, nt, E], _f32)
            xr = rtr_pool.tile([128, HT, TPC], _bf16)

            def emit_xr_load():
                nc.scalar.dma_start(
                    out=xr[:, :, :],
                    in_=xt_rtr[:, :].rearrange("(a p) n -> p a n", p=128))

            def emit_router_logits():
                for t in range(nt):
                    ps_l = psl_pool.tile([128, E], _f32, tag="psl")
                    for k in range(HT):
                        nc.tensor.matmul(
                            ps_l[:, :], xr[:, k, t * 128:(t + 1) * 128],
                            gw_sb[:, k, :],
                            start=(k == 0), stop=(k == HT - 1))
                    mneg = sm_pool.tile([128, 1], _f32)
                    nc.vector.tensor_reduce(
                        mneg[:, :], ps_l[:, :], axis=mybir.AxisListType.X,
                        op=mybir.AluOpType.max, negate=True)
                    expv = sm_pool.tile([128, E], _f32)
                    ssum = sm_pool.tile([128, 1], _f32)
                    nc.scalar.activation(
                        expv[:, :], ps_l[:, :],
                        mybir.ActivationFunctionType.Exp,
                        bias=mneg[:, 0:1], scale=1.0, accum_out=ssum[:, 0:1])
                    rs = sm_pool.tile([128, 1], _f32)
                    nc.vector.reciprocal(rs[:, :], ssum[:, :])
                    nc.vector.tensor_scalar_mul(
                        probs_all[:, t, :], expv[:, :], rs[:, 0:1])

            def emit_router_usage():
                # separate pass: by now the softmax chain has long finished,
                # so the PE never waits on it
                for t in range(nt):
                    nc.tensor.matmul(
                        ps_u[:, :], probs_all[:, t, :], ones[:, :],
                        start=(t == 0), stop=(t == nt - 1),
                        skip_group_check=True)
                u_sb = sm_pool.tile([E, 1], _f32)
                nc.vector.tensor_copy(u_sb[:, :], ps_u[:, :])
                nc.sync.dma_start(out=usage_p[:, :], in_=u_sb[:, :])

            w_sb = const_pool.tile([128, CAP + HT], _f32)
            nc.scalar.dma_start(out=w_sb[:, :], in_=w_bc[:, :])

            # ---- expert FFN over two half-passes of HALF tokens ----
            # Loop order maximizes stationary-weight reuse (one LDWEIGHTS per
            # 3 matmuls) while keeping g for one half resident in SBUF.
            def load_xc(hoff):
                # split the half's activation load into 4 pieces across two
                # DMA queues so the first matmuls can start early
                xc = xc_pool.tile([128, HT, HALF], _bf16, tag="xc")
                for piece in range(4):
                    eng = nc.sync if piece == 0 else nc.gpsimd
                    eng.dma_start(
                        out=xc[:, piece * 2:(piece + 1) * 2, :],
                        in_=xt_ffn[piece * 256:(piece + 1) * 256,
                                   hoff:hoff + HALF].rearrange(
                            "(a p) n -> p a n", p=128))
                return xc

            xc_next = load_xc(0)
            for hoff in (0, HALF):
                cs = _chunks(HALF)
                xc = xc_next
                g = g_pool.tile([128, IT, HALF], _bf16)
                # mm1: g[i] = gelu(up_w[:, i].T @ xc + up_b[i]); weight tiles
                # streamed in pairs of I-tiles for 512B DMA segments
                for ip in range(IT // 2):
                    if hoff == 0 and ip == 4:
                        emit_xr_load()
                    upt = up_pool.tile([128, HT, 256], _bf16)
                    eng = nc.sync if ip % 2 == 0 else nc.gpsimd
                    eng.dma_start(
                        out=upt[:, :, :],
                        in_=up_w[:, ip * 256:(ip + 1) * 256].rearrange(
                            "(a p) n -> p a n", p=128))
                    for sub in range(2):
                        i = ip * 2 + sub
                        ps1 = ps_pool.tile([128, HALF], _f32, tag="mmps")
                        for k in range(HT):
                            for (off, ncols) in cs:
                                nc.tensor.matmul(
                                    ps1[:, off:off + ncols],
                                    upt[:, k, sub * 128:(sub + 1) * 128],
                                    xc[:, k, off:off + ncols],
                                    start=(k == 0), stop=(k == HT - 1))
                        nc.scalar.activation(
                            g[:, i, :], ps1[:, :],
                            mybir.ActivationFunctionType.Gelu,
                            bias=upb_sb[:, i:i + 1], scale=1.0)
                if hoff == 0:
                    # fills the PE bubble while mm2 waits on the last gelu
                    emit_router_logits()
                # mm2: yT[h] = (down_w[:, h].T @ g + down_b[h]) * w
                for hp in range(HT // 2):
                    if hp == 1 and hoff == 0:
                        xc_next = load_xc(HALF)
                        emit_router_usage()
                    dwn = dwn_pool.tile([128, IT, 256], _bf16)
                    nc.scalar.dma_start(
                        out=dwn[:, :, :],
                        in_=down_w[:, hp * 256:(hp + 1) * 256].rearrange(
                            "(a p) n -> p a n", p=128))
                    for sub in range(2):
                        h = hp * 2 + sub
                        ps2 = ps_pool.tile([128, HALF], _f32, tag="mmps")
                        for i in range(IT):
                            for (off, ncols) in cs:
                                nc.tensor.matmul(
                                    ps2[:, off:off + ncols],
                                    dwn[:, i, sub * 128:(sub + 1) * 128],
                                    g[:, i, off:off + ncols],
                                    start=(i == 0), stop=(i == IT - 1))
                        yt = y_pool.tile([128, HALF], _f32)
                        nc.vector.scalar_tensor_tensor(
                            yt[:, :], ps2[:, :],
                            w_sb[:, CAP + h:CAP + h + 1],
                            w_sb[:, hoff:hoff + HALF],
                            op0=mybir.AluOpType.add,
                            op1=mybir.AluOpType.mult)
                        nc.gpsimd.dma_start(
                            out=y_out[h * 128:(h + 1) * 128,
                                      hoff:hoff + HALF],
                            in_=yt[:, :])


    nc.compile()
    return nc


def _get_compiled():
    global _COMPILED
    if _COMPILED is None:
        _COMPILED = _build()
    return _COMPILED


def _route(x2d, gate_w):
    """fp32 routing identical in structure to the jax reference."""
    logits = x2d @ gate_w                       # [TOK, E] fp32
    m = logits.max(-1, keepdims=True)
    p = np.exp(logits - m, dtype=np.float32)
    p = p / p.sum(-1, keepdims=True, dtype=np.float32)
    i1 = np.argmax(p, axis=-1)
    rows = np.arange(p.shape[0])
    p1 = p[rows, i1]
    pm = p.copy()
    pm[rows, i1] = -np.inf
    i2 = np.argmax(pm, axis=-1)
    p2 = p[rows, i2]
    den = p1 + p2
    return i1.astype(np.int32), i2.astype(np.int32), (p1 / den).astype(
        np.float32), (p2 / den).astype(np.float32)


def kernel(x, gate_w, up_w, up_b, down_w, down_b):
    x = np.asarray(x, dtype=np.float32)
    gate_w = np.asarray(gate_w, dtype=np.float32)
    up_w = np.asarray(up_w, dtype=np.float32)
    up_b = np.asarray(up_b, dtype=np.float32)
    down_w = np.asarray(down_w, dtype=np.float32)
    down_b = np.asarray(down_b, dtype=np.float32)

    x2d = x.reshape(TOK, H)
    i1, i2, w1, w2 = _route(x2d, gate_w)

    # dispatch lists per expert
    sel_idx, sel_w, overflow = [], [], []
    for e in range(E):
        sel = np.where((i1 == e) | (i2 == e))[0]
        w = np.where(i1[sel] == e, w1[sel], w2[sel]).astype(np.float32)
        if len(sel) > CAP:  # safety net; never hit for the spec'd inputs
            overflow.append((e, sel[CAP:], w[CAP:]))
            sel, w = sel[:CAP], w[:CAP]
        sel_idx.append(sel)
        sel_w.append(w)

    nc = _get_compiled()

    in_maps = []
    for e in range(E):
        sel, w = sel_idx[e], sel_w[e]
        n = len(sel)
        xt = np.zeros((H, CAP), dtype=BF16)
        xt[:, :n] = x2d[sel].T.astype(BF16)
        wb = np.zeros((128, CAP + HT), dtype=np.float32)
        wb[:, :n] = w[None, :]
        wb[:, CAP:] = down_b[e].reshape(HT, 128).T
        in_maps.append({
            "xt_ffn": xt,
            "up_w": up_w[e].astype(BF16),
            "down_w": down_w[e].astype(BF16),
            "up_b": np.ascontiguousarray(up_b[e].reshape(IT, 128).T),
            "w_bc": wb,
            "xt_rtr": np.ascontiguousarray(x2d[e * TPC:(e + 1) * TPC].T).astype(BF16),
            "gate_w": gate_w.astype(BF16),
        })

    res = run_bass_kernel_spmd(nc, in_maps, core_ids=list(range(E)))

    out = np.zeros((TOK, H), dtype=np.float32)
    usage = np.zeros(E, dtype=np.float32)
    for e in range(E):
        y = res.results[e]["y_out"]            # [H, CAP] f32, already *w
        n = len(sel_idx[e])
        out[sel_idx[e]] += y[:, :n].T
        usage += res.results[e]["usage_p"][:, 0]
    usage /= np.float32(TOK)

    # overflow tokens (only if capacity were ever exceeded): host fp32 FFN
    for (e, sel, w) in overflow:
        h1 = x2d[sel] @ up_w[e] + up_b[e]
        from scipy.special import erf
        g = 0.5 * h1 * (1.0 + erf(h1 / np.sqrt(2.0)))
        out[sel] += w[:, None] * (g.astype(np.float32) @ down_w[e] + down_b[e])

    loss = np.float32(E) * np.sum(usage.astype(np.float32) ** 2)
    top_expert = i1.reshape(B, S)
    return (out.reshape(B, S, H), np.float32(loss), usage,
            top_expert.astype(np.int32))


# revision 32
# speedup vs baseline: 1.0093x; 1.0019x over previous
"""MoE layer (B=4,S=2048,H=1024,I=4096,E=8,K=2) as a Bass/Tile kernel on 8
Trainium2 NeuronCores.

Strategy (expert parallelism, per the sharding hint):
  - Host computes the routing decision (softmax + top-2 over E=8) in fp32 as
    the *sharding* step, and dispatches each expert's tokens to the core that
    owns that expert (all-to-all realized at input-sharding time).
  - Core e holds expert e's FFN weights (bf16) and runs the expert FFN
    (x @ up_w -> +up_b -> gelu -> @ down_w -> +down_b -> * combine_weight)
    over its dispatched tokens (fixed capacity C, zero-padded).
  - Each core also runs the (replicated-weight) router on its 1/8 slice of
    tokens on the PE (bf16 matmul, fp32 softmax) to produce the expert-usage
    partial sums for the aux outputs.
  - Host combine: scatter-add per-core compact outputs back to [B,S,H],
    usage = mean of device partials, loss = E * sum(usage^2),
    top_expert = routing decision.

Self-contained: hardcodes shapes from the problem spec.
"""
import os
import numpy as np
import ml_dtypes
from contextlib import ExitStack

from concourse import bass, bacc, tile, mybir
from concourse.bass_utils import run_bass_kernel_spmd

BF16 = ml_dtypes.bfloat16


def _ensure_axon_hooks_shim():
    """bass_utils' axon trace path imports antenv.axon_hooks, which is absent
    in this image; provide a no-op shim so a stray BASS_TRACE=1 in the
    environment degrades to 'no trace' instead of crashing."""
    try:
        import antenv.axon_hooks  # noqa: F401
    except Exception:
        import sys
        import types
        try:
            import antenv
        except Exception:
            return
        mod = types.ModuleType("antenv.axon_hooks")
        _h = [None]
        mod.set_axon_ntff_profile_hook = lambda h: _h.__setitem__(0, h)
        mod.get_axon_ntff_profile_hook = lambda: _h[0]
        sys.modules["antenv.axon_hooks"] = mod
        antenv.axon_hooks = mod


_ensure_axon_hooks_shim()

# Problem shapes
B, S, H, I, E, TOPK = 4, 2048, 1024, 4096, 8, 2
TOK = B * S           # 8192 tokens
TPC = TOK // E        # 1024 tokens per core for the router slice
CAP = 2184            # per-expert token capacity (max observed count is 2182)
CHUNK = 512           # moving-operand/PSUM-bank tile over tokens

HT = H // 128         # 8  H tiles
IT = I // 128         # 32 I tiles

_f32 = mybir.dt.float32
_bf16 = mybir.dt.bfloat16

_COMPILED = None

HALF = CAP // 2       # tokens per half-pass


def _chunks(total):
    out = []
    off = 0
    while off < total:
        n = min(CHUNK, total - off)
        out.append((off, n))
        off += n
    return out


def _build():
    """Build + compile the single-core SPMD program (same program on all 8
    cores; per-core data differs via in_maps)."""
    nc = bacc.Bacc("TRN2", target_bir_lowering=False, debug=False)

    # ---- DRAM I/O ----
    xt_ffn = nc.dram_tensor("xt_ffn", [H, CAP], _bf16, kind="ExternalInput")
    up_w = nc.dram_tensor("up_w", [H, I], _bf16, kind="ExternalInput")
    down_w = nc.dram_tensor("down_w", [I, H], _bf16, kind="ExternalInput")
    up_b = nc.dram_tensor("up_b", [128, IT], _f32, kind="ExternalInput")
    w_bc = nc.dram_tensor("w_bc", [128, CAP + HT], _f32, kind="ExternalInput")
    xt_rtr = nc.dram_tensor("xt_rtr", [H, TPC], _bf16, kind="ExternalInput")
    gate_w = nc.dram_tensor("gate_w", [H, E], _bf16, kind="ExternalInput")

    y_out = nc.dram_tensor("y_out", [H, CAP], _f32, kind="ExternalOutput")
    usage_p = nc.dram_tensor("usage_p", [E, 1], _f32, kind="ExternalOutput")

    with tile.TileContext(nc) as tc:
        with ExitStack() as ctx:
            const_pool = ctx.enter_context(tc.tile_pool(name="const", bufs=1))
            up_pool = ctx.enter_context(tc.tile_pool(name="up", bufs=8))
            dwn_pool = ctx.enter_context(tc.tile_pool(name="dwn", bufs=2))
            # bufs=1: the slot dependency keeps half-1's activation load out of
            # the startup window (Tile would otherwise hoist it ahead of the
            # first weight pairs); it still prefetches ~200us before use
            xc_pool = ctx.enter_context(tc.tile_pool(name="xc", bufs=1))
            g_pool = ctx.enter_context(tc.tile_pool(name="g", bufs=1))
            y_pool = ctx.enter_context(tc.tile_pool(name="y", bufs=2))
            rtr_pool = ctx.enter_context(tc.tile_pool(name="rtr", bufs=1))
            sm_pool = ctx.enter_context(tc.tile_pool(name="sm", bufs=4))
            ps_pool = ctx.enter_context(
                tc.tile_pool(name="ps", bufs=2, space="PSUM"))
            psl_pool = ctx.enter_context(
                tc.tile_pool(name="psl", bufs=1, space="PSUM"))
            psu_pool = ctx.enter_context(
                tc.tile_pool(name="psu", bufs=1, space="PSUM"))

            # ---- constants ----
            ones = const_pool.tile([128, 1], _f32)
            nc.vector.memset(ones[:, :], 1.0)
            gw_sb = const_pool.tile([128, HT, E], _bf16)
            nc.scalar.dma_start(
                out=gw_sb[:, :, :],
                in_=gate_w[:, :].rearrange("(a p) e -> p a e", p=128))
            upb_sb = const_pool.tile([128, IT], _f32)
            nc.scalar.dma_start(out=upb_sb[:, :], in_=up_b[:, :])
            # ---- router over this core's 1/8 token slice (bf16 matmul,
            # fp32 softmax; only feeds expert_usage). Its input streams in on
            # the scalar queue while the FFN runs; its matmuls are emitted in
            # the natural mm1->mm2 bubble of the first FFN half-pass so they
            # never delay the FFN stream. ----
            ps_u = psu_pool.tile([E, 1], _f32)
            nt = TPC // 128
            probs_all = const_pool.tile([128, nt, E], _f32)
            xr = rtr_pool.tile([128, HT, TPC], _bf16)

            def emit_xr_load():
                nc.scalar.dma_start(
                    out=xr[:, :, :],
                    in_=xt_rtr[:, :].rearrange("(a p) n -> p a n", p=128))

            def emit_router_logits():
                for t in range(nt):
                    ps_l = psl_pool.tile([128, E], _f32, tag="psl")
                    for k in range(HT):
                        nc.tensor.matmul(
                            ps_l[:, :], xr[:, k, t * 128:(t + 1) * 128],
                            gw_sb[:, k, :],
                            start=(k == 0), stop=(k == HT - 1))
                    mneg = sm_pool.tile([128, 1], _f32)
                    nc.vector.tensor_reduce(
                        mneg[:, :], ps_l[:, :], axis=mybir.AxisListType.X,
                        op=mybir.AluOpType.max, negate=True)
                    expv = sm_pool.tile([128, E], _f32)
                    ssum = sm_pool.tile([128, 1], _f32)
                    nc.scalar.activation(
                        expv[:, :], ps_l[:, :],
                        mybir.ActivationFunctionType.Exp,
                        bias=mneg[:, 0:1], scale=1.0, accum_out=ssum[:, 0:1])
                    rs = sm_pool.tile([128, 1], _f32)
                    nc.vector.reciprocal(rs[:, :], ssum[:, :])
                    nc.vector.tensor_scalar_mul(
                        probs_all[:, t, :], expv[:, :], rs[:, 0:1])

            def emit_router_usage():
                # separate pass: by now the softmax chain has long finished,
                # so the PE never waits on it
                for t in range(nt):
                    nc.tensor.matmul(
                        ps_u[:, :], probs_all[:, t, :], ones[:, :],
                        start=(t == 0), stop=(t == nt - 1),
                        skip_group_check=True)
                u_sb = sm_pool.tile([E, 1], _f32)
                nc.vector.tensor_copy(u_sb[:, :], ps_u[:, :])
                nc.sync.dma_start(out=usage_p[:, :], in_=u_sb[:, :])

            w_sb = const_pool.tile([128, CAP + HT], _f32)
            nc.scalar.dma_start(out=w_sb[:, :], in_=w_bc[:, :])

            # ---- expert FFN over two half-passes of HALF tokens ----
            # Loop order maximizes stationary-weight reuse (one LDWEIGHTS per
            # 3 matmuls) while keeping g for one half resident in SBUF.
            def load_xc(hoff):
                # split the half's activation load into 4 pieces across two
                # DMA queues so the first matmuls can start early
                xc = xc_pool.tile([128, HT, HALF], _bf16, tag="xc")
                for piece in range(4):
                    eng = nc.sync if piece == 0 else nc.gpsimd
                    eng.dma_start(
                        out=xc[:, piece * 2:(piece + 1) * 2, :],
                        in_=xt_ffn[piece * 256:(piece + 1) * 256,
                                   hoff:hoff + HALF].rearrange(
                            "(a p) n -> p a n", p=128))
                return xc

            # hoist the very first up-weight pair to the head of the
            # gpsimd queue: the kernel's first LDWEIGHTS waits on it
            upt_first = up_pool.tile([128, HT, 256], _bf16, tag="upt")
            nc.gpsimd.dma_start(
                out=upt_first[:, :, :],
                in_=up_w[:, 0:256].rearrange("(a p) n -> p a n", p=128))

            xc_next = load_xc(0)
            for hoff in (0, HALF):
                cs = _chunks(HALF)
                xc = xc_next
                g = g_pool.tile([128, IT, HALF], _bf16)
                # mm1: g[i] = gelu(up_w[:, i].T @ xc + up_b[i]); weight tiles
                # streamed in pairs of I-tiles for 512B DMA segments
                for ip in range(IT // 2):
                    if hoff == 0 and ip == 4:
                        emit_xr_load()
                    if hoff == 0 and ip == 0:
                        upt = upt_first
                    else:
                        upt = up_pool.tile([128, HT, 256], _bf16, tag="upt")
                        eng = nc.sync if ip % 2 == 0 else nc.gpsimd
                        eng.dma_start(
                            out=upt[:, :, :],
                            in_=up_w[:, ip * 256:(ip + 1) * 256].rearrange(
                                "(a p) n -> p a n", p=128))
                    for sub in range(2):
                        i = ip * 2 + sub
                        ps1 = ps_pool.tile([128, HALF], _f32, tag="mmps")
                        for k in range(HT):
                            for (off, ncols) in cs:
                                nc.tensor.matmul(
                                    ps1[:, off:off + ncols],
                                    upt[:, k, sub * 128:(sub + 1) * 128],
                                    xc[:, k, off:off + ncols],
                                    start=(k == 0), stop=(k == HT - 1))
                        nc.scalar.activation(
                            g[:, i, :], ps1[:, :],
                            mybir.ActivationFunctionType.Gelu,
                            bias=upb_sb[:, i:i + 1], scale=1.0)
                if hoff == 0:
                    # fills the PE bubble while mm2 waits on the last gelu
                    emit_router_logits()
                # mm2: yT[h] = (down_w[:, h].T @ g + down_b[h]) * w
                for hp in range(HT // 2):
                    if hp == 1 and hoff == 0:
                        xc_next = load_xc(HALF)
                        emit_router_usage()
                    dwn = dwn_pool.tile([128, IT, 256], _bf16)
                    nc.scalar.dma_start(
                        out=dwn[:, :, :],
                        in_=down_w[:, hp * 256:(hp + 1) * 256].rearrange(
                            "(a p) n -> p a n", p=128))
                    for sub in range(2):
                        h = hp * 2 + sub
                        ps2 = ps_pool.tile([128, HALF], _f32, tag="mmps")
                        for i in range(IT):
                            for (off, ncols) in cs:
                                nc.tensor.matmul(
                                    ps2[:, off:off + ncols],
                                    dwn[:, i, sub * 128:(sub + 1) * 128],
                                    g[:, i, off:off + ncols],
                                    start=(i == 0), stop=(i == IT - 1))
                        yt = y_pool.tile([128, HALF], _f32)
                        nc.vector.scalar_tensor_tensor(
                            yt[:, :], ps2[:, :],
                            w_sb[:, CAP + h:CAP + h + 1],
                            w_sb[:, hoff:hoff + HALF],
                            op0=mybir.AluOpType.add,
                            op1=mybir.AluOpType.mult)
                        nc.gpsimd.dma_start(
                            out=y_out[h * 128:(h + 1) * 128,
                                      hoff:hoff + HALF],
                            in_=yt[:, :])


    nc.compile()
    return nc


def _get_compiled():
    global _COMPILED
    if _COMPILED is None:
        _COMPILED = _build()
    return _COMPILED


def _route(x2d, gate_w):
    """fp32 routing identical in structure to the jax reference."""
    logits = x2d @ gate_w                       # [TOK, E] fp32
    m = logits.max(-1, keepdims=True)
    p = np.exp(logits - m, dtype=np.float32)
    p = p / p.sum(-1, keepdims=True, dtype=np.float32)
    i1 = np.argmax(p, axis=-1)
    rows = np.arange(p.shape[0])
    p1 = p[rows, i1]
    pm = p.copy()
    pm[rows, i1] = -np.inf
    i2 = np.argmax(pm, axis=-1)
    p2 = p[rows, i2]
    den = p1 + p2
    return i1.astype(np.int32), i2.astype(np.int32), (p1 / den).astype(
        np.float32), (p2 / den).astype(np.float32)


def kernel(x, gate_w, up_w, up_b, down_w, down_b):
    x = np.asarray(x, dtype=np.float32)
    gate_w = np.asarray(gate_w, dtype=np.float32)
    up_w = np.asarray(up_w, dtype=np.float32)
    up_b = np.asarray(up_b, dtype=np.float32)
    down_w = np.asarray(down_w, dtype=np.float32)
    down_b = np.asarray(down_b, dtype=np.float32)

    x2d = x.reshape(TOK, H)
    i1, i2, w1, w2 = _route(x2d, gate_w)

    # dispatch lists per expert
    sel_idx, sel_w, overflow = [], [], []
    for e in range(E):
        sel = np.where((i1 == e) | (i2 == e))[0]
        w = np.where(i1[sel] == e, w1[sel], w2[sel]).astype(np.float32)
        if len(sel) > CAP:  # safety net; never hit for the spec'd inputs
            overflow.append((e, sel[CAP:], w[CAP:]))
            sel, w = sel[:CAP], w[:CAP]
        sel_idx.append(sel)
        sel_w.append(w)

    nc = _get_compiled()

    in_maps = []
    for e in range(E):
        sel, w = sel_idx[e], sel_w[e]
        n = len(sel)
        xt = np.zeros((H, CAP), dtype=BF16)
        xt[:, :n] = x2d[sel].T.astype(BF16)
        wb = np.zeros((128, CAP + HT), dtype=np.float32)
        wb[:, :n] = w[None, :]
        wb[:, CAP:] = down_b[e].reshape(HT, 128).T
        in_maps.append({
            "xt_ffn": xt,
            "up_w": up_w[e].astype(BF16),
            "down_w": down_w[e].astype(BF16),
            "up_b": np.ascontiguousarray(up_b[e].reshape(IT, 128).T),
            "w_bc": wb,
            "xt_rtr": np.ascontiguousarray(x2d[e * TPC:(e + 1) * TPC].T).astype(BF16),
            "gate_w": gate_w.astype(BF16),
        })

    res = run_bass_kernel_spmd(nc, in_maps, core_ids=list(range(E)))

    out = np.zeros((TOK, H), dtype=np.float32)
    usage = np.zeros(E, dtype=np.float32)
    for e in range(E):
        y = res.results[e]["y_out"]            # [H, CAP] f32, already *w
        n = len(sel_idx[e])
        out[sel_idx[e]] += y[:, :n].T
        usage += res.results[e]["usage_p"][:, 0]
    usage /= np.float32(TOK)

    # overflow tokens (only if capacity were ever exceeded): host fp32 FFN
    for (e, sel, w) in overflow:
        h1 = x2d[sel] @ up_w[e] + up_b[e]
        from scipy.special import erf
        g = 0.5 * h1 * (1.0 + erf(h1 / np.sqrt(2.0)))
        out[sel] += w[:, None] * (g.astype(np.float32) @ down_w[e] + down_b[e])

    loss = np.float32(E) * np.sum(usage.astype(np.float32) ** 2)
    top_expert = i1.reshape(B, S)
    return (out.reshape(B, S, H), np.float32(loss), usage,
            top_expert.astype(np.int32))
